# revision 29
# baseline (speedup 1.0000x reference)
"""Trainium2 Bass kernel for block-local (chunked) attention.

Problem: x:(4,4096,1024) f32. qkv = x @ w_qkv.T; block-local attention with
chunk=64 inside each head (16 heads, dim_head 64); out proj w_out + b_out.

Strategy (8 cores, SPMD):
  - Shard the 16384 flattened tokens into 8 contiguous shards of 2048
    (chunk-aligned, so blocks never cross shards).
  - Host pre-transposes x and the weights so every DMA is contiguous and
    every matmul operand has the contraction dim on partitions.
  - Per core: qkv projection (bf16 matmuls at N=1024, fp32 PSUM
    accumulate), block attention with the softmax reduction done ON the
    partition axis via a block-identity matmul (sum + broadcast in one PE
    op), final projection, pipelined with the Tile framework.

Layouts on device (P=128 partitions always first):
  xT     [128, 8, T]     bf16   xT[p,kc,t]  = x_shard[t, kc*128+p]
  wqkvT  [128, 8, 3072]  bf16   [p,kc,f]    = w_qkv[f, kc*128+p]
  woutT  [128, 8, 1024]  bf16   [p,hp,e]    = w_out[e, hp*128+p]
  consts [128, 128]      bf16   [:, :64]=upper-half ones, [:, 64:]=lower-half
  yT     [8, 128, T]     f32    yT[ec,p,t]  = y_shard[t, ec*128+p]

HW gotcha baked in below: matmuls whose stationary operands live at
different base partitions (row groups 0 vs 64) must never target the same
PSUM bank — that crashes the device. Scores matmuls are therefore grouped
by head parity into separate PSUM tiles.
"""

import os
import sys

for _p in ("/opt/trn_rl_repo", "/root/.axon_site/_ro/trn_rl_repo"):
    if os.path.isdir(_p) and _p not in sys.path:
        sys.path.append(_p)

import numpy as np
import ml_dtypes

import concourse.bass as bass
from concourse import bacc
from concourse import mybir
from concourse import tile

BF16 = mybir.dt.bfloat16
F32 = mybir.dt.float32
BF16_NP = ml_dtypes.bfloat16

P = 128
KC = 8            # contraction chunks for dim=1024
HEADS = 16
DH = 64
CHUNK = 64
INNER = HEADS * DH            # 1024
DIM = 1024
N_CORES = 8
ST = 128                      # tokens per attention subtile
SCALE = DH ** -0.5


def build_body(tc, yT, xT, wqkvT, woutT, consts, T):
    """Emit the whole per-core program into TileContext tc."""
    nc = tc.nc
    GT = min(512, T)          # tokens per group
    G = T // GT
    NST = GT // ST            # subtiles per group
    import contextlib
    ctx = contextlib.ExitStack()

    # --- SBUF pools -----------------------------------------------------
    wpool = ctx.enter_context(tc.tile_pool(name="w", bufs=1))
    xpool = ctx.enter_context(tc.tile_pool(name="x", bufs=3))
    qkpool = ctx.enter_context(tc.tile_pool(name="qk", bufs=2))
    vpool = ctx.enter_context(tc.tile_pool(name="v", bufs=3))
    epool = ctx.enter_context(tc.tile_pool(name="e", bufs=2))
    rbpool = ctx.enter_context(tc.tile_pool(name="rb", bufs=2))
    enpool = ctx.enter_context(tc.tile_pool(name="en", bufs=1))
    apool = ctx.enter_context(tc.tile_pool(name="a", bufs=2))
    ypool = ctx.enter_context(tc.tile_pool(name="y", bufs=2))

    # --- PSUM pools: 2 + 2 + 2 + 1 + 1 = 8 banks ------------------------
    ps_big = ctx.enter_context(tc.tile_pool(name="ps_big", bufs=2, space="PSUM"))
    ps_v = ctx.enter_context(tc.tile_pool(name="ps_v", bufs=2, space="PSUM"))
    ps_sc = ctx.enter_context(tc.tile_pool(name="ps_sc", bufs=2, space="PSUM"))
    ps_sm = ctx.enter_context(tc.tile_pool(name="ps_sm", bufs=1, space="PSUM"))
    ps_at = ctx.enter_context(tc.tile_pool(name="ps_at", bufs=1, space="PSUM"))

    # --- startup DMA schedule ------------------------------------------
    # The sync DMA ring is one FIFO processed in issue order, and each
    # dma_start costs ~0.8us of trigger programming on the sync queue. So:
    # few triggers, ordered so the first fc chain's needs (x group 0 + the
    # first 256 wqkv feature cols) complete first.
    x_first = xpool.tile([P, KC * (GT if T >= 512 else T)], BF16, tag="x",
                         name="x_first")
    GT0 = min(512, T)
    x_first3 = x_first[:].rearrange("p (k t) -> p k t", k=KC)
    wqkv_sb = wpool.tile([P, KC * 3072], BF16, tag="wqkv")
    wq3 = wqkv_sb[:].rearrange("p (k f) -> p k f", k=KC)
    # group-0 q/k run kc-outer, so interleave (x kc, q-weights kc) transfers
    # in exactly consumption order; the first matmul then waits on ~380 KiB,
    # not 5 MiB.
    for kc in range(KC):
        nc.sync.dma_start(x_first3[:, kc, :], xT[:, kc, 0:GT0])
        nc.sync.dma_start(wq3[:, kc, 0:1024], wqkvT[:, kc, 0:1024])
    for kc in range(KC):
        nc.sync.dma_start(wq3[:, kc, 1024:2048], wqkvT[:, kc, 1024:2048])
    nc.sync.dma_start(wq3[:, :, 2048:3072], wqkvT[:, :, 2048:3072])
    consts_sb = wpool.tile([P, 128], BF16, tag="consts")
    nc.sync.dma_start(consts_sb[:], consts[:, :])
    # w_out is only needed by the first final-projection (~1/3 into the
    # kernel) — defer its DMA so the startup loads get the full bandwidth.
    wout_sb = wpool.tile([P, KC * 1024], BF16, tag="wout")
    wout_loaded = [False]

    def load_wout():
        if not wout_loaded[0]:
            nc.sync.dma_start(
                wout_sb[:].rearrange("p (k e) -> p k e", k=KC), woutT[:, :, :]
            )
            wout_loaded[0] = True

    # --- persistent pre-zeroed block-diagonal stationaries --------------
    # dk[g%2]: per-group diag-k, [128, 8 chunks x (8 pairs x 128)].  Block
    # (gc, pr) is a [128,128] block-diagonal stationary: [0:64, 0:64] =
    # k_even[d, tk in gc], [64:128, 64:128] = k_odd[d, tk in gc].  Off-diag
    # zeros persist from the one-time memset; the k projection copies only
    # rewrite the diagonal quadrants.  One 64-col matmul against q (both
    # heads' d stacked on partitions) then yields both heads' scores —
    # halving the streamed columns vs per-head 64-partition matmuls.
    dk_ring = []
    for i in range(2):
        t = enpool.tile([P, 8 * 8 * 128], BF16, tag=f"dk{i}", name=f"dk{i}")
        nc.gpsimd.memset(t[:], 0.0)
        dk_ring.append(t)
    # dv ring: per-subtile diag-v, [128, 2 chunks x (8 pairs x 128)], same
    # block-diagonal trick for attn@v ([0:64,0:64] = v_even[tk in c, d],
    # [64:128,64:128] = v_odd[tk in c, d]).
    dv_ring = []
    for i in range(3):
        t = enpool.tile([P, 2 * 8 * 128], BF16, tag=f"dv{i}", name=f"dv{i}")
        nc.gpsimd.memset(t[:], 0.0)
        dv_ring.append(t)
    # vsw: partition-half-swapped copy of v (SBUF->SBUF DMA) feeding the
    # diag-v quadrants whose tokens live on the wrong partition half.
    vsw_ring = [enpool.tile([P, INNER], BF16, tag=f"vs{i}", name=f"vsw{i}")
                for i in range(3)]

    st_idx = 0
    for g in range(G):
        # ---- x tile for this group ------------------------------------
        if g == 0:
            x_t = x_first
        else:
            x_t = xpool.tile([P, KC * GT], BF16, tag="x")
            xt3 = x_t[:].rearrange("p (k t) -> p k t", k=KC)
            nc.sync.dma_start(xt3[:, :, :], xT[:, :, g * GT:(g + 1) * GT])

        # ---- q/k projections: out layout [feat, tok] -------------------
        q_sb = qkpool.tile([P, 8 * GT], BF16, tag="q")
        dk = dk_ring[g % 2]
        dk4 = dk[:].rearrange("p (c r t) -> p c r t", c=8, r=8)
        attn_sb = apool.tile([P, NST * 8 * 128], BF16, tag="attn",
                             name=f"attn_{g}")

        def copy_k_diag(fc2, src_ps):
            # scatter one k feature-pair into its block-diagonal slots
            # (upper quadrant <- even head, lower <- odd head) across all
            # 8 chunks of the group; off-diagonal zeros persist.
            s3 = src_ps[:].rearrange("p (c t) -> p c t", c=8)
            nc.vector.tensor_copy(dk4[0:64, :, fc2, 0:64], s3[0:64, :, :])
            nc.scalar.copy(dk4[64:128, :, fc2, 64:128], s3[64:128, :, :])
        if g == 0:
            # Boot schedule: kc-outer across all 8 PSUM banks, so the first
            # matmul needs only x chunk 0 + the kc-0 q-weight slice, and the
            # PE ramps while the rest of the startup traffic streams in.
            for half in range(2):
                banks = [ps_big.tile([P, GT], F32, tag="big", name=f"bb{half}0"),
                         ps_big.tile([P, GT], F32, tag="big", name=f"bb{half}1"),
                         ps_v.tile([P, 512], F32, tag="v", name=f"bv{half}0"),
                         ps_v.tile([P, 512], F32, tag="v", name=f"bv{half}1"),
                         ps_sc.tile([P, 512], F32, tag="sc", name=f"bs{half}0"),
                         ps_sc.tile([P, 512], F32, tag="sc", name=f"bs{half}1"),
                         ps_sm.tile([P, 512], F32, tag="sm", name=f"bm{half}"),
                         ps_at.tile([P, 512], F32, tag="at", name=f"ba{half}")]
                for kc in range(KC):
                    for fc8 in range(8):
                        fc = half * 8 + fc8
                        nc.tensor.matmul(
                            banks[fc8][:],
                            lhsT=wqkv_sb[:, kc * 3072 + fc * 128:
                                         kc * 3072 + fc * 128 + 128],
                            rhs=x_t[:, kc * GT:(kc + 1) * GT],
                            start=(kc == 0),
                            stop=(kc == KC - 1),
                        )
                for fc8 in range(8):
                    if half == 0:
                        sl = q_sb[:, fc8 * GT:(fc8 + 1) * GT]
                        if fc8 % 2 == 0:
                            nc.vector.tensor_copy(sl, banks[fc8][:])
                        else:
                            nc.scalar.copy(sl, banks[fc8][:])
                    else:
                        copy_k_diag(fc8, banks[fc8])
        else:
            for fc in range(16):
                qk_ps = ps_big.tile([P, GT], F32, tag="big")
                for kc in range(KC):
                    nc.tensor.matmul(
                        qk_ps[:],
                        lhsT=wqkv_sb[:, kc * 3072 + fc * 128: kc * 3072 + fc * 128 + 128],
                        rhs=x_t[:, kc * GT:(kc + 1) * GT],
                        start=(kc == 0),
                        stop=(kc == KC - 1),
                    )
                if fc < 8:
                    sl = q_sb[:, fc * GT:(fc + 1) * GT]
                    if fc % 2 == 0:
                        nc.vector.tensor_copy(sl, qk_ps[:])
                    else:
                        nc.scalar.copy(sl, qk_ps[:])
                else:
                    copy_k_diag(fc - 8, qk_ps)

        # ---- final projection helper -----------------------------------
        # (s0, s1, t0, t1): subtile range plus an optional sub-subtile token
        # window (t-range within the single subtile s0 when s1 == s0 + 1).
        # Last group: rotate final-proj chains over the otherwise-idle
        # v/sm/at PSUM banks too (all of them only ever host
        # base-0-stationary matmuls), so a chain never stalls on the
        # previous chain's PSUM->SBUF copy.
        a3 = attn_sb[:].rearrange("p (s h t) -> p s h t", s=NST, h=8)
        if g == G - 1:
            pool_cycle = [(ps_big, "big"), (ps_v, "v"), (ps_sm, "sm"),
                          (ps_big, "big"), (ps_v, "v"), (ps_at, "at")]
        else:
            pool_cycle = [(ps_big, "big")]
        pist = [0]

        def emit_final(s0, s1, t0, t1):
            load_wout()
            ht = (s1 - s0 - 1) * ST + (t1 - t0)
            for ec in range(8):
                pool, ptag = pool_cycle[pist[0] % len(pool_cycle)]
                pist[0] += 1
                f_ps = pool.tile([P, GT], F32, tag=ptag,
                                 name=f"fps_{g}_{s0}_{t0}_{ec}")
                for hp in range(KC):
                    nc.tensor.matmul(
                        f_ps[:, 0:ht],
                        lhsT=wout_sb[:, hp * 1024 + ec * 128: hp * 1024 + ec * 128 + 128],
                        rhs=a3[:, s0:s1, hp, t0:t1] if t1 - t0 < ST
                        else a3[:, s0:s1, hp, :],
                        start=(hp == 0),
                        stop=(hp == KC - 1),
                    )
                if ec == 0:
                    y_sb = ypool.tile([P, KC * GT], BF16, tag="y",
                                      name=f"y_{g}_{s0}_{t0}")
                    y3 = y_sb[:].rearrange("p (e t) -> p e t", e=KC)
                if ec % 2 == 0:
                    nc.vector.tensor_copy(y3[:, ec, 0:ht], f_ps[:, 0:ht])
                else:
                    nc.scalar.copy(y3[:, ec, 0:ht], f_ps[:, 0:ht])
            # one batched DMA per token slice (8 triggers -> 1: the sync
            # queue's ~0.8us per trigger was the kernel-exit drain)
            base = g * GT + s0 * ST + t0
            nc.sync.dma_start(yT[:, :, base: base + ht], y3[:, :, 0:ht])

        # ---- per 128-token subtile: v projection + attention -----------
        for st in range(NST):
            # v projection: out layout [tok, feat], one N=1024 matmul per kc
            v_sb = vpool.tile([P, INNER], BF16, tag="v", name=f"v_{g}_{st}")
            for half in range(2):
                v_ps = ps_v.tile([P, 512], F32, tag="v")
                for kc in range(KC):
                    nc.tensor.matmul(
                        v_ps[:],
                        lhsT=x_t[:, kc * GT + st * ST: kc * GT + st * ST + ST],
                        rhs=wqkv_sb[:, kc * 3072 + 2048 + half * 512:
                                    kc * 3072 + 2048 + (half + 1) * 512],
                        start=(kc == 0),
                        stop=(kc == KC - 1),
                    )
                nc.vector.tensor_copy(v_sb[:, half * 512:(half + 1) * 512], v_ps[:])

            # ---- diag-v construction: one partition-half-swapped copy of
            # v (SBUF->SBUF DMA), then scatter the four quadrant families
            # into the pre-zeroed block-diagonal tiles.
            vsw = vsw_ring[st_idx % 3]
            dv = dv_ring[st_idx % 3]
            st_idx += 1
            v16 = v_sb[:].rearrange("p (h x) -> p h x", h=16)
            w16 = vsw[:].rearrange("p (h x) -> p h x", h=16)
            nc.sync.dma_start(w16[64:128, 1:16:2, :], v16[0:64, 1:16:2, :])
            nc.sync.dma_start(w16[0:64, 0:16:2, :], v16[64:128, 0:16:2, :])
            dv4 = dv[:].rearrange("p (c r t) -> p c r t", c=2, r=8)
            nc.vector.tensor_copy(dv4[0:64, 0, :, 0:64], v16[0:64, 0:16:2, :])
            nc.scalar.copy(dv4[64:128, 0, :, 64:128], w16[64:128, 1:16:2, :])
            nc.scalar.copy(dv4[0:64, 1, :, 0:64], w16[0:64, 0:16:2, :])
            nc.vector.tensor_copy(dv4[64:128, 1, :, 64:128], v16[64:128, 1:16:2, :])

            a4 = attn_sb[:].rearrange("p (s r t) -> p s r t", s=NST, r=8)
            for c in range(2):        # 64-token chunks of this subtile
                gc = st * 2 + c
                # scores: one block-diag stationary per head pair, moving =
                # q for both heads (d stacked on partitions), 64 tq cols.
                # Out rows 0:64 = even head's tk, 64:128 = odd head's tk.
                sc_ps = ps_sc.tile([P, 512], F32, tag="sc")
                for pr in range(8):
                    nc.tensor.matmul(
                        sc_ps[:, pr * 64:(pr + 1) * 64],
                        lhsT=dk[:, gc * 1024 + pr * 128: gc * 1024 + (pr + 1) * 128],
                        rhs=q_sb[:, pr * GT + st * ST + c * 64:
                                 pr * GT + st * ST + c * 64 + 64],
                        start=True, stop=True,
                    )
                # exp (scale folded in); all values valid — no garbage halves
                e_sb = epool.tile([P, 512], BF16, tag="e")
                nc.scalar.activation(
                    e_sb[:], sc_ps[:], mybir.ActivationFunctionType.Exp,
                    scale=SCALE,
                )
                # denominators: block-diag ones matmul sums each partition
                # half separately and broadcasts within the half.
                sm_ps = ps_sm.tile([P, 512], F32, tag="sm")
                nc.tensor.matmul(sm_ps[:], lhsT=consts_sb[:, 0:128],
                                 rhs=e_sb[:], start=True, stop=True)
                rb = rbpool.tile([P, 512], F32, tag="rb")
                nc.vector.reciprocal_approx_fast(out=rb[:], in_=sm_ps[:])
                # attn @ v on unnormalized exp; diag-v zeros kill the
                # cross-head terms.  Out rows 0:64 = even head's d, 64:128 =
                # odd head's d — matching the final projection layout.
                # alternate with a ps_big bank (idle during the middle) so
                # chunk c+1's attn@v never waits on chunk c's normalize-mul
                if gc % 2 == 0:
                    av_ps = ps_at.tile([P, 512], F32, tag="at")
                else:
                    av_ps = ps_big.tile([P, GT], F32, tag="big",
                                        name=f"avb_{g}_{gc}")
                for pr in range(8):
                    nc.tensor.matmul(
                        av_ps[:, pr * 64:(pr + 1) * 64],
                        lhsT=dv[:, c * 1024 + pr * 128: c * 1024 + (pr + 1) * 128],
                        rhs=e_sb[:, pr * 64:(pr + 1) * 64],
                        start=True, stop=True,
                    )
                # fused normalize + copy-out (rb rows already align per half)
                av3 = av_ps[:].rearrange("p (r t) -> p r t", r=8)
                rb3 = rb[:].rearrange("p (r t) -> p r t", r=8)
                nc.vector.tensor_mul(
                    a4[:, st, :, c * 64:c * 64 + 64], av3[:, :, :], rb3[:, :, :]
                )

        # ---- final projection ------------------------------------------
        # For the last group there is no following work to hide the
        # attention→final serialization, so split it into progressively
        # smaller token slices.
        if g == G - 1 and NST > 2:
            emit_final(0, NST - 2, 0, ST)
            emit_final(NST - 2, NST - 1, 0, ST)
            emit_final(NST - 1, NST, 0, ST // 2)
            emit_final(NST - 1, NST, ST // 2, ST)
        else:
            emit_final(0, NST, 0, ST)

    ctx.close()


def build_nc(T):
    nc = bacc.Bacc("TRN2", target_bir_lowering=False, debug=False)
    xT = nc.dram_tensor("xT", [P, KC, T], BF16, kind="ExternalInput").ap()
    wqkvT = nc.dram_tensor("wqkvT", [P, KC, 3072], BF16, kind="ExternalInput").ap()
    woutT = nc.dram_tensor("woutT", [P, KC, 1024], BF16, kind="ExternalInput").ap()
    consts = nc.dram_tensor("consts", [P, 128], BF16, kind="ExternalInput").ap()
    yT = nc.dram_tensor("yT", [P, KC, T], BF16, kind="ExternalOutput").ap()
    with tile.TileContext(nc) as tc:
        build_body(tc, yT, xT, wqkvT, woutT, consts, T)
    nc.compile()
    return nc


def make_consts():
    c = np.zeros((P, 128), dtype=BF16_NP)
    c[0:64, 0:64] = 1
    c[64:128, 64:128] = 1
    return c


def prep_inputs(x, w_qkv, w_out, T):
    """Host-side shard + transpose + cast. Returns in_maps list for SPMD."""
    tok = x.shape[0] * x.shape[1]
    flat = np.ascontiguousarray(x.reshape(tok, DIM))
    wqkvT = np.ascontiguousarray(
        w_qkv.T.reshape(KC, P, 3072).transpose(1, 0, 2)
    ).astype(BF16_NP)
    woutT = np.ascontiguousarray(
        w_out.T.reshape(KC, P, 1024).transpose(1, 0, 2)
    ).astype(BF16_NP)
    consts = make_consts()
    n_cores = tok // T
    in_maps = []
    for c in range(n_cores):
        shard = flat[c * T:(c + 1) * T]           # [T, 1024]
        xTc = np.ascontiguousarray(
            shard.T.reshape(KC, P, T).transpose(1, 0, 2)
        ).astype(BF16_NP)
        in_maps.append({"xT": xTc, "wqkvT": wqkvT, "woutT": woutT,
                        "consts": consts})
    return in_maps


def postprocess(results, b_out, bshape, T):
    outs = []
    for r in results:
        yT = np.asarray(r["yT"]).astype(np.float32)   # [128, 8, T] (bf16)
        outs.append(yT.transpose(1, 0, 2).reshape(DIM, T).T)  # [T, 1024]
    y = np.concatenate(outs, axis=0)                  # [tok, 1024]
    y = y + np.asarray(b_out, dtype=np.float32)[None, :]
    return y.reshape(*bshape, DIM)


_CACHED = {}


def kernel(x, w_qkv, w_out, b_out):
    from concourse.bass_utils import run_bass_kernel_spmd

    x = np.asarray(x)
    b, n, _ = x.shape
    T = (b * n) // N_CORES
    if T not in _CACHED:
        _CACHED[T] = build_nc(T)
    nc = _CACHED[T]
    in_maps = prep_inputs(x, np.asarray(w_qkv), np.asarray(w_out), T)
    res = run_bass_kernel_spmd(nc, in_maps, list(range(N_CORES)))
    return postprocess(res.results, b_out, (b, n), T)


if __name__ == "__main__":
    nc = build_nc(2048)
    print("built ok")



# revision 31
# speedup vs baseline: 1.0039x; 1.0039x over previous
"""Trainium2 Bass kernel for block-local (chunked) attention.

Problem: x:(4,4096,1024) f32. qkv = x @ w_qkv.T; block-local attention with
chunk=64 inside each head (16 heads, dim_head 64); out proj w_out + b_out.

Strategy (8 cores, SPMD):
  - Shard the 16384 flattened tokens into 8 contiguous shards of 2048
    (chunk-aligned, so blocks never cross shards).
  - Host pre-transposes x and the weights so every DMA is contiguous and
    every matmul operand has the contraction dim on partitions.
  - Per core: qkv projection (bf16 matmuls at N=1024, fp32 PSUM
    accumulate), block attention with the softmax reduction done ON the
    partition axis via a block-identity matmul (sum + broadcast in one PE
    op), final projection, pipelined with the Tile framework.

Layouts on device (P=128 partitions always first):
  xT     [128, 8, T]     bf16   xT[p,kc,t]  = x_shard[t, kc*128+p]
  wqkvT  [128, 8, 3072]  bf16   [p,kc,f]    = w_qkv[f, kc*128+p]
  woutT  [128, 8, 1024]  bf16   [p,hp,e]    = w_out[e, hp*128+p]
  consts [128, 128]      bf16   [:, :64]=upper-half ones, [:, 64:]=lower-half
  yT     [8, 128, T]     f32    yT[ec,p,t]  = y_shard[t, ec*128+p]

HW gotcha baked in below: matmuls whose stationary operands live at
different base partitions (row groups 0 vs 64) must never target the same
PSUM bank — that crashes the device. Scores matmuls are therefore grouped
by head parity into separate PSUM tiles.
"""

import os
import sys

for _p in ("/opt/trn_rl_repo", "/root/.axon_site/_ro/trn_rl_repo"):
    if os.path.isdir(_p) and _p not in sys.path:
        sys.path.append(_p)

import numpy as np
import ml_dtypes

import concourse.bass as bass
from concourse import bacc
from concourse import mybir
from concourse import tile

BF16 = mybir.dt.bfloat16
F32 = mybir.dt.float32
BF16_NP = ml_dtypes.bfloat16

P = 128
KC = 8            # contraction chunks for dim=1024
HEADS = 16
DH = 64
CHUNK = 64
INNER = HEADS * DH            # 1024
DIM = 1024
N_CORES = 8
ST = 128                      # tokens per attention subtile
SCALE = DH ** -0.5


def build_body(tc, yT, xT, wqkvT, woutT, consts, T):
    """Emit the whole per-core program into TileContext tc."""
    nc = tc.nc
    GT = min(512, T)          # tokens per group
    G = T // GT
    NST = GT // ST            # subtiles per group
    import contextlib
    ctx = contextlib.ExitStack()

    # --- SBUF pools -----------------------------------------------------
    wpool = ctx.enter_context(tc.tile_pool(name="w", bufs=1))
    xpool = ctx.enter_context(tc.tile_pool(name="x", bufs=3))
    qkpool = ctx.enter_context(tc.tile_pool(name="qk", bufs=2))
    vpool = ctx.enter_context(tc.tile_pool(name="v", bufs=3))
    epool = ctx.enter_context(tc.tile_pool(name="e", bufs=2))
    rbpool = ctx.enter_context(tc.tile_pool(name="rb", bufs=2))
    enpool = ctx.enter_context(tc.tile_pool(name="en", bufs=1))
    apool = ctx.enter_context(tc.tile_pool(name="a", bufs=2))
    ypool = ctx.enter_context(tc.tile_pool(name="y", bufs=2))

    # --- PSUM pools: 2 + 2 + 2 + 1 + 1 = 8 banks ------------------------
    ps_big = ctx.enter_context(tc.tile_pool(name="ps_big", bufs=2, space="PSUM"))
    ps_v = ctx.enter_context(tc.tile_pool(name="ps_v", bufs=2, space="PSUM"))
    ps_sc = ctx.enter_context(tc.tile_pool(name="ps_sc", bufs=2, space="PSUM"))
    ps_sm = ctx.enter_context(tc.tile_pool(name="ps_sm", bufs=1, space="PSUM"))
    ps_at = ctx.enter_context(tc.tile_pool(name="ps_at", bufs=1, space="PSUM"))

    # --- startup DMA schedule ------------------------------------------
    # The sync DMA ring is one FIFO processed in issue order, and each
    # dma_start costs ~0.8us of trigger programming on the sync queue. So:
    # few triggers, ordered so the first fc chain's needs (x group 0 + the
    # first 256 wqkv feature cols) complete first.
    x_first = xpool.tile([P, KC * (GT if T >= 512 else T)], BF16, tag="x",
                         name="x_first")
    GT0 = min(512, T)
    x_first3 = x_first[:].rearrange("p (k t) -> p k t", k=KC)
    wqkv_sb = wpool.tile([P, KC * 3072], BF16, tag="wqkv")
    wq3 = wqkv_sb[:].rearrange("p (k f) -> p k f", k=KC)
    # group-0 q/k run kc-outer, so interleave (x kc, q-weights kc) transfers
    # in exactly consumption order; the first matmul then waits on ~380 KiB,
    # not 5 MiB.
    for kc in range(KC):
        nc.sync.dma_start(x_first3[:, kc, :], xT[:, kc, 0:GT0])
        nc.sync.dma_start(wq3[:, kc, 0:1024], wqkvT[:, kc, 0:1024])
    for kc in range(KC):
        nc.sync.dma_start(wq3[:, kc, 1024:2048], wqkvT[:, kc, 1024:2048])
    nc.sync.dma_start(wq3[:, :, 2048:3072], wqkvT[:, :, 2048:3072])
    consts_sb = wpool.tile([P, 128], BF16, tag="consts")
    nc.sync.dma_start(consts_sb[:], consts[:, :])
    # w_out is only needed by the first final-projection (~1/3 into the
    # kernel) — defer its DMA so the startup loads get the full bandwidth.
    wout_sb = wpool.tile([P, KC * 1024], BF16, tag="wout")
    wout_loaded = [False]

    def load_wout():
        if not wout_loaded[0]:
            nc.sync.dma_start(
                wout_sb[:].rearrange("p (k e) -> p k e", k=KC), woutT[:, :, :]
            )
            wout_loaded[0] = True

    # --- persistent pre-zeroed block-diagonal stationaries --------------
    # dk[g%2]: per-group diag-k, [128, 8 chunks x (8 pairs x 128)].  Block
    # (gc, pr) is a [128,128] block-diagonal stationary: [0:64, 0:64] =
    # k_even[d, tk in gc], [64:128, 64:128] = k_odd[d, tk in gc].  Off-diag
    # zeros persist from the one-time memset; the k projection copies only
    # rewrite the diagonal quadrants.  One 64-col matmul against q (both
    # heads' d stacked on partitions) then yields both heads' scores —
    # halving the streamed columns vs per-head 64-partition matmuls.
    dk_ring = []
    for i in range(2):
        t = enpool.tile([P, 8 * 8 * 128], BF16, tag=f"dk{i}", name=f"dk{i}")
        nc.gpsimd.memset(t[:], 0.0)
        dk_ring.append(t)
    # dv ring: per-subtile diag-v, [128, 2 chunks x (8 pairs x 128)], same
    # block-diagonal trick for attn@v ([0:64,0:64] = v_even[tk in c, d],
    # [64:128,64:128] = v_odd[tk in c, d]).
    dv_ring = []
    for i in range(3):
        t = enpool.tile([P, 2 * 8 * 128], BF16, tag=f"dv{i}", name=f"dv{i}")
        nc.gpsimd.memset(t[:], 0.0)
        dv_ring.append(t)
    # vsw: partition-half-swapped copy of v (SBUF->SBUF DMA) feeding the
    # diag-v quadrants whose tokens live on the wrong partition half.
    vsw_ring = [enpool.tile([P, INNER], BF16, tag=f"vs{i}", name=f"vsw{i}")
                for i in range(3)]

    st_idx = 0
    for g in range(G):
        # ---- x tile for this group ------------------------------------
        if g == 0:
            x_t = x_first
        else:
            x_t = xpool.tile([P, KC * GT], BF16, tag="x")
            xt3 = x_t[:].rearrange("p (k t) -> p k t", k=KC)
            nc.sync.dma_start(xt3[:, :, :], xT[:, :, g * GT:(g + 1) * GT])

        # ---- q/k projections: out layout [feat, tok] -------------------
        q_sb = qkpool.tile([P, 8 * GT], BF16, tag="q")
        dk = dk_ring[g % 2]
        dk4 = dk[:].rearrange("p (c r t) -> p c r t", c=8, r=8)
        attn_sb = apool.tile([P, NST * 8 * 128], BF16, tag="attn",
                             name=f"attn_{g}")

        def copy_k_diag(fc2, src_ps):
            # scatter one k feature-pair into its block-diagonal slots
            # (upper quadrant <- even head, lower <- odd head) across all
            # 8 chunks of the group; off-diagonal zeros persist.
            s3 = src_ps[:].rearrange("p (c t) -> p c t", c=8)
            nc.vector.tensor_copy(dk4[0:64, :, fc2, 0:64], s3[0:64, :, :])
            nc.scalar.copy(dk4[64:128, :, fc2, 64:128], s3[64:128, :, :])
        if g == 0:
            # Boot schedule: kc-outer across all 8 PSUM banks, so the first
            # matmul needs only x chunk 0 + the kc-0 q-weight slice, and the
            # PE ramps while the rest of the startup traffic streams in.
            for half in range(2):
                banks = [ps_big.tile([P, GT], F32, tag="big", name=f"bb{half}0"),
                         ps_big.tile([P, GT], F32, tag="big", name=f"bb{half}1"),
                         ps_v.tile([P, 512], F32, tag="v", name=f"bv{half}0"),
                         ps_v.tile([P, 512], F32, tag="v", name=f"bv{half}1"),
                         ps_sc.tile([P, 512], F32, tag="sc", name=f"bs{half}0"),
                         ps_sc.tile([P, 512], F32, tag="sc", name=f"bs{half}1"),
                         ps_sm.tile([P, 512], F32, tag="sm", name=f"bm{half}"),
                         ps_at.tile([P, 512], F32, tag="at", name=f"ba{half}")]
                for kc in range(KC):
                    for fc8 in range(8):
                        fc = half * 8 + fc8
                        nc.tensor.matmul(
                            banks[fc8][:],
                            lhsT=wqkv_sb[:, kc * 3072 + fc * 128:
                                         kc * 3072 + fc * 128 + 128],
                            rhs=x_t[:, kc * GT:(kc + 1) * GT],
                            start=(kc == 0),
                            stop=(kc == KC - 1),
                        )
                for fc8 in range(8):
                    if half == 0:
                        sl = q_sb[:, fc8 * GT:(fc8 + 1) * GT]
                        if fc8 % 2 == 0:
                            nc.vector.tensor_copy(sl, banks[fc8][:])
                        else:
                            nc.scalar.copy(sl, banks[fc8][:])
                    else:
                        copy_k_diag(fc8, banks[fc8])
        else:
            for fc in range(16):
                qk_ps = ps_big.tile([P, GT], F32, tag="big")
                for kc in range(KC):
                    nc.tensor.matmul(
                        qk_ps[:],
                        lhsT=wqkv_sb[:, kc * 3072 + fc * 128: kc * 3072 + fc * 128 + 128],
                        rhs=x_t[:, kc * GT:(kc + 1) * GT],
                        start=(kc == 0),
                        stop=(kc == KC - 1),
                    )
                if fc < 8:
                    sl = q_sb[:, fc * GT:(fc + 1) * GT]
                    if fc % 2 == 0:
                        nc.vector.tensor_copy(sl, qk_ps[:])
                    else:
                        nc.scalar.copy(sl, qk_ps[:])
                else:
                    copy_k_diag(fc - 8, qk_ps)

        # ---- final projection helper -----------------------------------
        # (s0, s1, t0, t1): subtile range plus an optional sub-subtile token
        # window (t-range within the single subtile s0 when s1 == s0 + 1).
        # Last group: rotate final-proj chains over the otherwise-idle
        # v/sm/at PSUM banks too (all of them only ever host
        # base-0-stationary matmuls), so a chain never stalls on the
        # previous chain's PSUM->SBUF copy.
        a3 = attn_sb[:].rearrange("p (s h t) -> p s h t", s=NST, h=8)
        if g == G - 1:
            pool_cycle = [(ps_big, "big"), (ps_v, "v"), (ps_sm, "sm"),
                          (ps_big, "big"), (ps_v, "v"), (ps_at, "at")]
        else:
            pool_cycle = [(ps_big, "big")]
        pist = [0]

        def emit_final(s0, s1, t0, t1):
            load_wout()
            ht = (s1 - s0 - 1) * ST + (t1 - t0)
            for ec in range(8):
                pool, ptag = pool_cycle[pist[0] % len(pool_cycle)]
                pist[0] += 1
                f_ps = pool.tile([P, GT], F32, tag=ptag,
                                 name=f"fps_{g}_{s0}_{t0}_{ec}")
                for hp in range(KC):
                    nc.tensor.matmul(
                        f_ps[:, 0:ht],
                        lhsT=wout_sb[:, hp * 1024 + ec * 128: hp * 1024 + ec * 128 + 128],
                        rhs=a3[:, s0:s1, hp, t0:t1] if t1 - t0 < ST
                        else a3[:, s0:s1, hp, :],
                        start=(hp == 0),
                        stop=(hp == KC - 1),
                    )
                if ec == 0:
                    y_sb = ypool.tile([P, KC * GT], BF16, tag="y",
                                      name=f"y_{g}_{s0}_{t0}")
                    y3 = y_sb[:].rearrange("p (e t) -> p e t", e=KC)
                if ec % 2 == 0:
                    nc.vector.tensor_copy(y3[:, ec, 0:ht], f_ps[:, 0:ht])
                else:
                    nc.scalar.copy(y3[:, ec, 0:ht], f_ps[:, 0:ht])
            # one batched DMA per token slice, on the SCALAR engine's DMA
            # ring: a separate hardware queue, so the output never waits
            # behind x-prefetch / v-swap traffic on the sync ring.
            base = g * GT + s0 * ST + t0
            nc.scalar.dma_start(yT[:, :, base: base + ht], y3[:, :, 0:ht])

        # ---- per 128-token subtile: v projection + attention -----------
        for st in range(NST):
            # v projection: out layout [tok, feat], one N=1024 matmul per kc
            v_sb = vpool.tile([P, INNER], BF16, tag="v", name=f"v_{g}_{st}")
            for half in range(2):
                v_ps = ps_v.tile([P, 512], F32, tag="v")
                for kc in range(KC):
                    nc.tensor.matmul(
                        v_ps[:],
                        lhsT=x_t[:, kc * GT + st * ST: kc * GT + st * ST + ST],
                        rhs=wqkv_sb[:, kc * 3072 + 2048 + half * 512:
                                    kc * 3072 + 2048 + (half + 1) * 512],
                        start=(kc == 0),
                        stop=(kc == KC - 1),
                    )
                nc.vector.tensor_copy(v_sb[:, half * 512:(half + 1) * 512], v_ps[:])

            # ---- diag-v construction: one partition-half-swapped copy of
            # v (SBUF->SBUF DMA), then scatter the four quadrant families
            # into the pre-zeroed block-diagonal tiles.
            vsw = vsw_ring[st_idx % 3]
            dv = dv_ring[st_idx % 3]
            st_idx += 1
            v16 = v_sb[:].rearrange("p (h x) -> p h x", h=16)
            w16 = vsw[:].rearrange("p (h x) -> p h x", h=16)
            nc.sync.dma_start(w16[64:128, 1:16:2, :], v16[0:64, 1:16:2, :])
            nc.sync.dma_start(w16[0:64, 0:16:2, :], v16[64:128, 0:16:2, :])
            dv4 = dv[:].rearrange("p (c r t) -> p c r t", c=2, r=8)
            nc.vector.tensor_copy(dv4[0:64, 0, :, 0:64], v16[0:64, 0:16:2, :])
            nc.scalar.copy(dv4[64:128, 0, :, 64:128], w16[64:128, 1:16:2, :])
            nc.scalar.copy(dv4[0:64, 1, :, 0:64], w16[0:64, 0:16:2, :])
            nc.vector.tensor_copy(dv4[64:128, 1, :, 64:128], v16[64:128, 1:16:2, :])

            a4 = attn_sb[:].rearrange("p (s r t) -> p s r t", s=NST, r=8)
            for c in range(2):        # 64-token chunks of this subtile
                gc = st * 2 + c
                # scores: one block-diag stationary per head pair, moving =
                # q for both heads (d stacked on partitions), 64 tq cols.
                # Out rows 0:64 = even head's tk, 64:128 = odd head's tk.
                sc_ps = ps_sc.tile([P, 512], F32, tag="sc")
                for pr in range(8):
                    nc.tensor.matmul(
                        sc_ps[:, pr * 64:(pr + 1) * 64],
                        lhsT=dk[:, gc * 1024 + pr * 128: gc * 1024 + (pr + 1) * 128],
                        rhs=q_sb[:, pr * GT + st * ST + c * 64:
                                 pr * GT + st * ST + c * 64 + 64],
                        start=True, stop=True,
                    )
                # exp (scale folded in); all values valid — no garbage halves
                e_sb = epool.tile([P, 512], BF16, tag="e")
                nc.scalar.activation(
                    e_sb[:], sc_ps[:], mybir.ActivationFunctionType.Exp,
                    scale=SCALE,
                )
                # denominators: block-diag ones matmul sums each partition
                # half separately and broadcasts within the half.
                sm_ps = ps_sm.tile([P, 512], F32, tag="sm")
                nc.tensor.matmul(sm_ps[:], lhsT=consts_sb[:, 0:128],
                                 rhs=e_sb[:], start=True, stop=True)
                rb = rbpool.tile([P, 512], F32, tag="rb")
                nc.vector.reciprocal_approx_fast(out=rb[:], in_=sm_ps[:])
                # attn @ v on unnormalized exp; diag-v zeros kill the
                # cross-head terms.  Out rows 0:64 = even head's d, 64:128 =
                # odd head's d — matching the final projection layout.
                av_ps = ps_at.tile([P, 512], F32, tag="at")
                for pr in range(8):
                    nc.tensor.matmul(
                        av_ps[:, pr * 64:(pr + 1) * 64],
                        lhsT=dv[:, c * 1024 + pr * 128: c * 1024 + (pr + 1) * 128],
                        rhs=e_sb[:, pr * 64:(pr + 1) * 64],
                        start=True, stop=True,
                    )
                # fused normalize + copy-out (rb rows already align per half)
                av3 = av_ps[:].rearrange("p (r t) -> p r t", r=8)
                rb3 = rb[:].rearrange("p (r t) -> p r t", r=8)
                nc.vector.tensor_mul(
                    a4[:, st, :, c * 64:c * 64 + 64], av3[:, :, :], rb3[:, :, :]
                )

        # ---- final projection ------------------------------------------
        # For the last group there is no following work to hide the
        # attention→final serialization, so split it into progressively
        # smaller token slices.
        if g == G - 1 and NST > 2:
            emit_final(0, NST - 2, 0, ST)
            emit_final(NST - 2, NST - 1, 0, ST)
            emit_final(NST - 1, NST, 0, ST // 2)
            emit_final(NST - 1, NST, ST // 2, ST)
        else:
            emit_final(0, NST, 0, ST)

    ctx.close()


def build_nc(T):
    nc = bacc.Bacc("TRN2", target_bir_lowering=False, debug=False)
    xT = nc.dram_tensor("xT", [P, KC, T], BF16, kind="ExternalInput").ap()
    wqkvT = nc.dram_tensor("wqkvT", [P, KC, 3072], BF16, kind="ExternalInput").ap()
    woutT = nc.dram_tensor("woutT", [P, KC, 1024], BF16, kind="ExternalInput").ap()
    consts = nc.dram_tensor("consts", [P, 128], BF16, kind="ExternalInput").ap()
    yT = nc.dram_tensor("yT", [P, KC, T], BF16, kind="ExternalOutput").ap()
    with tile.TileContext(nc) as tc:
        build_body(tc, yT, xT, wqkvT, woutT, consts, T)
    nc.compile()
    return nc


def make_consts():
    c = np.zeros((P, 128), dtype=BF16_NP)
    c[0:64, 0:64] = 1
    c[64:128, 64:128] = 1
    return c


def prep_inputs(x, w_qkv, w_out, T):
    """Host-side shard + transpose + cast. Returns in_maps list for SPMD."""
    tok = x.shape[0] * x.shape[1]
    flat = np.ascontiguousarray(x.reshape(tok, DIM))
    wqkvT = np.ascontiguousarray(
        w_qkv.T.reshape(KC, P, 3072).transpose(1, 0, 2)
    ).astype(BF16_NP)
    woutT = np.ascontiguousarray(
        w_out.T.reshape(KC, P, 1024).transpose(1, 0, 2)
    ).astype(BF16_NP)
    consts = make_consts()
    n_cores = tok // T
    in_maps = []
    for c in range(n_cores):
        shard = flat[c * T:(c + 1) * T]           # [T, 1024]
        xTc = np.ascontiguousarray(
            shard.T.reshape(KC, P, T).transpose(1, 0, 2)
        ).astype(BF16_NP)
        in_maps.append({"xT": xTc, "wqkvT": wqkvT, "woutT": woutT,
                        "consts": consts})
    return in_maps


def postprocess(results, b_out, bshape, T):
    outs = []
    for r in results:
        yT = np.asarray(r["yT"]).astype(np.float32)   # [128, 8, T] (bf16)
        outs.append(yT.transpose(1, 0, 2).reshape(DIM, T).T)  # [T, 1024]
    y = np.concatenate(outs, axis=0)                  # [tok, 1024]
    y = y + np.asarray(b_out, dtype=np.float32)[None, :]
    return y.reshape(*bshape, DIM)


_CACHED = {}


def kernel(x, w_qkv, w_out, b_out):
    from concourse.bass_utils import run_bass_kernel_spmd

    x = np.asarray(x)
    b, n, _ = x.shape
    T = (b * n) // N_CORES
    if T not in _CACHED:
        _CACHED[T] = build_nc(T)
    nc = _CACHED[T]
    in_maps = prep_inputs(x, np.asarray(w_qkv), np.asarray(w_out), T)
    res = run_bass_kernel_spmd(nc, in_maps, list(range(N_CORES)))
    return postprocess(res.results, b_out, (b, n), T)


if __name__ == "__main__":
    nc = build_nc(2048)
    print("built ok")

